# revision 1
# baseline (speedup 1.0000x reference)
"""Causal MHA on 8 trn2 NeuronCores.

Sharding: core c handles batch b = c // 4 and head group g = c % 4
(heads 4g..4g+3).  Megatron-style TP: W_kqv column-split per head
group, W_proj row-split; the row-parallel all-reduce (sum of the 4
head-group partials per batch) happens on the host at gather time.

Per-core program (all matmuls in float32r = full-rate fp32):
  - qT,kT produced directly in [feat, T] layout (lhsT=W tiles, rhs=xT
    tiles), v in [T, feat] layout (lhsT=xT tiles, rhs=W tiles), so no
    on-device transposes are ever needed.
  - scores computed transposed, sT[k,q], two heads packed into the PE
    array rows (K=64 each) via base-partition 0/64 -> concurrent MMs.
  - exp on the scalar engine straight out of PSUM (1/sqrt(hd) folded
    into Wq on the host); causal mask via gpsimd affine_select.
  - PV uses ones-augmented V ([k,65] lhsT) so PSUM row 64 accumulates
    the softmax denominator Z alongside the 64 output dims.
  - normalization: DVE reciprocal of the Z row, broadcast across 64
    partitions with a K=1 ones-matmul, ACT copy + DVE multiply.
  - emission interleaves QKV(j+1) and proj(j-1) matmuls into
    attention(j)'s exp-gated stream so the PE stays dense and warm
    (HAM stays at K=8/8); activations are per-j tiles so the
    interleaved phases share no tiles (no false dependencies).
"""

import sys

sys.path.insert(0, "/opt/trn_rl_repo")

import numpy as np

import concourse.bass as bass
import concourse.tile as tile
from concourse import bacc, mybir

F32 = mybir.dt.float32
F32R = mybir.dt.float32r

B, T, D = 2, 2048, 1024
H, HD = 16, 64
N_CORES = 8
HPG = H // (N_CORES // B)  # heads per group = 4
GF = HPG * HD  # per-group feature width = 256
DT = 512  # t/q tile width
KT = 128  # k tile width
NJ = T // DT  # 4
ND = D // 128  # 8 contraction chunks


def build_program(num_devices=N_CORES):
    nc = bacc.Bacc(
        "TRN2", target_bir_lowering=False, debug=False, num_devices=num_devices
    )
    xT_d = nc.dram_tensor("xT", [D, T], F32R, kind="ExternalInput")
    wq_d = nc.dram_tensor("wq", [D, GF], F32R, kind="ExternalInput")
    wk_d = nc.dram_tensor("wk", [D, GF], F32R, kind="ExternalInput")
    wv_d = nc.dram_tensor("wv", [D, GF], F32R, kind="ExternalInput")
    wp_d = nc.dram_tensor("wp", [GF, D], F32R, kind="ExternalInput")
    bq_d = nc.dram_tensor("bq", [128, 2], F32, kind="ExternalInput")
    bk_d = nc.dram_tensor("bk", [128, 2], F32, kind="ExternalInput")
    bv_d = nc.dram_tensor("bv", [1, GF], F32R, kind="ExternalInput")
    ones_d = nc.dram_tensor("ones", [128, 128], F32R, kind="ExternalInput")
    msk_d = nc.dram_tensor("msk", [128, HPG, DT], F32R, kind="ExternalInput")
    sel_d = nc.dram_tensor("sel", [128, 256], F32R, kind="ExternalInput")
    y_d = nc.dram_tensor("y", [T, D], F32, kind="ExternalOutput")

    with tile.TileContext(nc) as tc:
        with (
            tc.tile_pool(name="singles", bufs=1) as singles,
            tc.tile_pool(name="ea", bufs=6) as e_pool,
            tc.tile_pool(name="rz", bufs=2) as rz_pool,
            tc.tile_pool(name="xt", bufs=2) as xt_pool,
            tc.tile_pool(name="ysb", bufs=3) as y_pool,
            tc.tile_pool(name="tr", bufs=4, space="PSUM") as tr_pool,
            tc.tile_pool(name="pv", bufs=4, space="PSUM") as pv_pool,
        ):
            # ---- weights / constants resident in SBUF ----
            wq_sb = singles.tile([128, ND, GF], F32R)
            wk_sb = singles.tile([128, ND, GF], F32R)
            wv_sb = singles.tile([128, ND, GF], F32R)
            wp_sb = singles.tile([128, 2, D], F32R)
            bq_sb = singles.tile([128, 2], F32)
            bk_sb = singles.tile([128, 2], F32)
            bv_sb = singles.tile([1, GF], F32R)
            ones_sb = singles.tile([128, 128], F32R)
            msk_sb = singles.tile([128, HPG, DT], F32R)
            sel_sb = singles.tile([128, 256], F32R)

            wq_v = wq_d.ap().rearrange("(o p) c -> p o c", p=128)
            wk_v = wk_d.ap().rearrange("(o p) c -> p o c", p=128)
            wv_v = wv_d.ap().rearrange("(o p) c -> p o c", p=128)
            wp_v = wp_d.ap().rearrange("(o p) n -> p o n", p=128)
            xT_v = xT_d.ap().rearrange("(o p) t -> p o t", p=128)

            # per-j activation tiles (distinct tiles -> no false deps
            # between interleaved phases)
            qT_t = [singles.tile([128, 2, DT], F32R, tag=f"qT{j}", name=f"qT{j}") for j in range(NJ)]
            kT_t = [singles.tile([128, 2, DT], F32R, tag=f"kT{j}", name=f"kT{j}") for j in range(NJ)]
            v_t = [
                singles.tile([128, DT // KT, HPG, HD + 1], F32R, tag=f"v{j}", name=f"v{j}")
                for j in range(NJ)
            ]
            o_t = [singles.tile([128, 2, DT], F32R, tag=f"oT{j}", name=f"oT{j}") for j in range(NJ)]
            xt_t = {}

            def new_xt(j):
                xt_t[j] = xt_pool.tile([128, ND, DT], F32R, tag="xt", name=f"xt{j}")

            # ---- early, chunked loads: first QKV matmul needs only
            # wq[d=0] + xt0[d=0] ----
            new_xt(0)
            for d in range(ND):
                nc.sync.dma_start(xt_t[0][:, d, :], xT_v[:, d, 0:DT]).annotate("ld:xt0")
                nc.sync.dma_start(wq_sb[:, d, :], wq_v[:, d, :]).annotate("ld:wq")
                nc.sync.dma_start(wk_sb[:, d, :], wk_v[:, d, :]).annotate("ld:wk")
                nc.sync.dma_start(wv_sb[:, d, :], wv_v[:, d, :]).annotate("ld:wv")
            nc.sync.dma_start(bq_sb, bq_d.ap()).annotate("ld:b")
            nc.sync.dma_start(bk_sb, bk_d.ap()).annotate("ld:b")
            nc.sync.dma_start(bv_sb, bv_d.ap()).annotate("ld:b")
            nc.sync.dma_start(ones_sb, ones_d.ap()).annotate("ld:b")
            nc.sync.dma_start(msk_sb, msk_d.ap()).annotate("ld:b")
            nc.sync.dma_start(sel_sb, sel_d.ap()).annotate("ld:b")
            for ch in range(2):
                nc.sync.dma_start(wp_sb[:, ch, :], wp_v[:, ch, :]).annotate("ld:wp")
            ones1 = ones_sb[0:1, :]
            ones64 = ones_sb[0:1, 0:64]
            for j in range(NJ):
                nc.vector.tensor_copy(
                    out=v_t[j][:, :, :, HD],
                    in_=ones_sb[:, 0 : DT // KT * HPG].rearrange(
                        "p (a b) -> p a b", a=DT // KT
                    ),
                ).annotate("v:ones")

            def qkv_closures(j):
                """QKV production for t-tile j as a list of closures."""
                cls = []
                if j > 0:

                    def ldx(j=j):
                        new_xt(j)
                        for d in range(ND):
                            nc.sync.dma_start(
                                xt_t[j][:, d, :],
                                xT_v[:, d, j * DT : (j + 1) * DT],
                            ).annotate(f"ld:xt{j}")

                    cls.append(ldx)

                for w_sb, b_sb, dst in (
                    (wq_sb, bq_sb, qT_t[j]),
                    (wk_sb, bk_sb, kT_t[j]),
                ):

                    def qk(j=j, w_sb=w_sb, b_sb=b_sb, dst=dst):
                        for ch in range(2):
                            ps = tr_pool.tile([128, DT], F32, tag="tr")
                            csl = slice(ch * 128, ch * 128 + 128)
                            for d in range(ND):
                                nc.tensor.matmul(
                                    ps,
                                    w_sb[:, d, csl],
                                    xt_t[j][:, d, :],
                                    start=(d == 0),
                                    stop=(d == ND - 1),
                                ).annotate("mm:qk")
                            nc.vector.tensor_scalar_add(
                                out=dst[:, ch, :],
                                in0=ps,
                                scalar1=b_sb[:, ch : ch + 1],
                            ).annotate("cp:qk")

                    cls.append(qk)

                for t_ in range(DT // KT):

                    def vv(j=j, t_=t_):
                        ps = tr_pool.tile([128, GF], F32, tag="tr")
                        ssl = slice(t_ * 128, t_ * 128 + 128)
                        for d in range(ND):
                            nc.tensor.matmul(
                                ps,
                                xt_t[j][:, d, ssl],
                                wv_sb[:, d, :],
                                start=(d == 0),
                                stop=False,
                            ).annotate("mm:v")
                        nc.tensor.matmul(
                            ps, ones1, bv_sb, start=False, stop=True
                        ).annotate("mm:vb")
                        nc.vector.tensor_copy(
                            out=v_t[j][:, t_, :, 0:HD],
                            in_=ps.rearrange("p (h c) -> p h c", c=HD),
                        ).annotate("cp:v")

                    cls.append(vv)
                return cls

            def attn_closures(j):
                """Attention for q-tile j: per-(pair,kt) closures plus a
                normalize closure per pair."""
                q0 = j * DT
                nk = (q0 + DT) // KT
                cls = []
                pvs = []
                zz = rz_pool.tile([128, DT], F32, tag="zz", name=f"zz{j}")
                nc.vector.memset(zz, 1.0)
                for pair in range(2):
                    pvA = pv_pool.tile([HD + 1, DT], F32, tag="pv")
                    pvB = pv_pool.tile([HD + 1, DT], F32, tag="pv")

                    for kt in range(nk):

                        def step(j=j, pair=pair, kt=kt, pvA=pvA, pvB=pvB, nk=nk):
                            jk, km = kt // (DT // KT), kt % (DT // KT)
                            ksl = slice(km * KT, km * KT + KT)
                            psA = tr_pool.tile([128, DT], F32, tag="tr")
                            psB = tr_pool.tile([128, DT], F32, tag="tr")
                            nc.tensor.matmul(
                                psA,
                                kT_t[jk][0:64, pair, ksl],
                                qT_t[j][0:64, pair, :],
                                start=True,
                                stop=True,
                            ).annotate("mm:s")
                            nc.tensor.matmul(
                                psB,
                                kT_t[jk][64:128, pair, ksl],
                                qT_t[j][64:128, pair, :],
                                start=True,
                                stop=True,
                            ).annotate("mm:s")
                            eA = e_pool.tile([128, DT], F32R, tag="eA")
                            eB = e_pool.tile([128, DT], F32R, tag="eB")
                            nc.scalar.activation(
                                out=eA, in_=psA, func=mybir.ActivationFunctionType.Exp
                            ).annotate("exp")
                            nc.scalar.activation(
                                out=eB, in_=psB, func=mybir.ActivationFunctionType.Exp
                            ).annotate("exp")
                            m = kt - (DT // KT) * j
                            if m >= 0:  # diagonal tile: causal mask
                                for ee in (eA, eB):
                                    nc.vector.tensor_mul(
                                        out=ee, in0=ee, in1=msk_sb[:, m, :]
                                    ).annotate("mask")
                            nc.tensor.matmul(
                                pvA,
                                v_t[jk][:, km, 2 * pair, :],
                                eA,
                                start=(kt == 0),
                                stop=(kt == nk - 1),
                            ).annotate("mm:pv")
                            nc.tensor.matmul(
                                pvB,
                                v_t[jk][:, km, 2 * pair + 1, :],
                                eB,
                                start=(kt == 0),
                                stop=(kt == nk - 1),
                            ).annotate("mm:pv")

                        cls.append(step)

                    def zcp(j=j, pair=pair, pvA=pvA, pvB=pvB, zz=zz):
                        # copy Z rows and the unnormalized outputs NOW so the
                        # pv PSUM banks release before the (slow) reciprocal
                        for half, pv in ((0, pvA), (1, pvB)):
                            row = 32 * (2 * pair + half)
                            nc.vector.tensor_copy(
                                out=zz[row : row + 1, :],
                                in_=pv[HD : HD + 1, :],
                            ).annotate("zcp")
                            osl = o_t[j][half * 64 : half * 64 + 64, pair, :]
                            nc.scalar.activation(
                                out=osl,
                                in_=pv[0:HD, :],
                                func=mybir.ActivationFunctionType.Copy,
                            ).annotate("cp:o")

                    cls.append(zcp)
                    pvs.append((pvA, pvB))

                def norm(j=j, zz=zz):
                    zr = rz_pool.tile([128, DT], F32R, tag="zr")
                    with nc.allow_low_precision(reason="f32r rz for bcast"):
                        nc.vector.reciprocal(
                            out=zr[0:97, :], in_=zz[0:97, :]
                        ).annotate("rz")
                    for pair in range(2):
                        bz = tr_pool.tile([128, DT], F32, tag="tr")
                        nc.tensor.matmul(
                            bz,
                            sel_sb[0:97, pair * 128 : (pair + 1) * 128],
                            zr[0:97, :],
                            start=True,
                            stop=True,
                        ).annotate("mm:bz")
                        for half in range(2):
                            osl = o_t[j][half * 64 : half * 64 + 64, pair, :]
                            nc.vector.tensor_mul(
                                out=osl, in0=osl, in1=bz[half * 64 : half * 64 + 64, :]
                            ).annotate("mul:o")

                return cls, norm

            def proj_closures(j):
                cls = []
                for t_ in range(DT // KT):

                    def pj(j=j, t_=t_):
                        t0 = j * DT + t_ * 128
                        lsl = slice(t_ * 128, t_ * 128 + 128)
                        ysb = y_pool.tile([128, D], F32, tag="y")
                        for n in range(2):
                            ps = tr_pool.tile([128, DT], F32, tag="tr")
                            nsl = slice(n * DT, n * DT + DT)
                            for ch in range(2):
                                nc.tensor.matmul(
                                    ps,
                                    o_t[j][:, ch, lsl],
                                    wp_sb[:, ch, nsl],
                                    start=(ch == 0),
                                    stop=(ch == 1),
                                ).annotate("mm:p")
                            nc.vector.tensor_copy(out=ysb[:, nsl], in_=ps).annotate(
                                "cp:y"
                            )
                        nc.sync.dma_start(y_d.ap()[t0 : t0 + 128, :], ysb).annotate(
                            "st:y"
                        )

                    cls.append(pj)
                return cls

            # ---- emission: QKV(0) plain, then per j interleave
            # attention(j) with QKV(j+1) + proj(j-1) ----
            for c in qkv_closures(0):
                c()
            prev_norm = None
            for j in range(NJ):
                attn, norm = attn_closures(j)
                fill = []
                if prev_norm is not None:
                    fill.append(prev_norm)
                if j + 1 < NJ:
                    fill += qkv_closures(j + 1)
                if j >= 1:
                    fill += proj_closures(j - 1)
                prev_norm = norm
                done = 0
                for i, c in enumerate(attn):
                    c()
                    want = (i + 1) * len(fill) // len(attn)
                    while done < want:
                        fill[done]()
                        done += 1
                while done < len(fill):
                    fill[done]()
                    done += 1
            prev_norm()
            for c in proj_closures(NJ - 1):
                c()

    return nc


def shard_inputs(x, W_kqv, b_kqv, W_proj, b_proj):
    """Build the 8 per-core input maps (host-side layout transforms)."""
    scale = 1.0 / np.sqrt(np.float32(HD))
    in_maps = []
    for c in range(N_CORES):
        b = c // (N_CORES // B)
        g = c % (N_CORES // B)
        gsl = slice(g * GF, (g + 1) * GF)
        wq = np.ascontiguousarray(W_kqv[:, gsl]) * scale
        wk = np.ascontiguousarray(W_kqv[:, D + g * GF : D + (g + 1) * GF])
        wv = np.ascontiguousarray(W_kqv[:, 2 * D + g * GF : 2 * D + (g + 1) * GF])
        bq = (b_kqv[gsl] * scale).reshape(2, 128).T
        bk = b_kqv[D + g * GF : D + (g + 1) * GF].reshape(2, 128).T
        bv = b_kqv[2 * D + g * GF : 2 * D + (g + 1) * GF].reshape(1, GF)
        in_maps.append(
            {
                "xT": np.ascontiguousarray(x[b].T).astype(np.float32),
                "wq": wq.astype(np.float32),
                "wk": wk.astype(np.float32),
                "wv": wv.astype(np.float32),
                "wp": np.ascontiguousarray(W_proj[gsl, :]).astype(np.float32),
                "bq": np.ascontiguousarray(bq).astype(np.float32),
                "bk": np.ascontiguousarray(bk).astype(np.float32),
                "bv": bv.astype(np.float32),
                "ones": np.ones((128, 128), dtype=np.float32),
                "msk": _mask_tiles(),
                "sel": _sel_tiles(),
            }
        )
    return in_maps


def _sel_tiles():
    sel = np.zeros((128, 256), dtype=np.float32)
    for p in range(2):
        for c in range(128):
            sel[32 * (2 * p + (c >= 64)), p * 128 + c] = 1.0
    return sel


def _mask_tiles():
    i = np.arange(128)[:, None]
    jj = np.arange(DT)[None, :]
    return np.stack(
        [(jj >= i + KT * m).astype(np.float32) for m in range(HPG)], axis=1
    )


def gather_outputs(results, b_proj):
    out = np.zeros((B, T, D), dtype=np.float32)
    for c in range(N_CORES):
        out[c // (N_CORES // B)] += results[c]["y"]
    out += b_proj[None, None, :].astype(np.float32)
    return out


_NC_CACHE = {}


def _get_program():
    if "nc" not in _NC_CACHE:
        nc = build_program()
        nc.finalize()  # runs Bacc passes (reg alloc, wait splitting)
        _NC_CACHE["nc"] = nc
    return _NC_CACHE["nc"]


def run(inputs, trace=False):
    """Run on the 8 NeuronCores; returns (out, BassKernelResults)."""
    from concourse import bass_utils

    nc = _get_program()
    in_maps = shard_inputs(**inputs)
    res = bass_utils.run_bass_kernel_spmd(
        nc,
        in_maps,
        core_ids=list(range(N_CORES)),
        trace=trace,
        trace_cores=list(range(N_CORES)) if trace else None,
    )
    out = gather_outputs(res.results, inputs["b_proj"])
    return out, res


def kernel(**inputs):
    out, _ = run(inputs, trace=False)
    return out



# revision 2
# speedup vs baseline: 1.2883x; 1.2883x over previous
"""Causal MHA on 8 trn2 NeuronCores.

Sharding: core c handles batch b = c // 4 and head group g = c % 4
(heads 4g..4g+3).  Megatron-style TP: W_kqv column-split per head
group, W_proj row-split; the row-parallel all-reduce (sum of the 4
head-group partials per batch) happens on the host at gather time.

Per-core program (bf16 matmul operands, fp32 PSUM accumulation):
  - qT,kT produced directly in [feat, T] layout (lhsT=W tiles, rhs=xT
    tiles), v in [T, feat] layout, so no on-device transposes.
  - scores computed transposed, sT[k,q]; two heads packed into the PE
    array rows (K=64 each) via base-partition 0/64 -> concurrent MMs
    into the two banks of one [128,2,512] f32 PSUM tile.
  - causal trim: diagonal k-tiles only compute the valid q-range
    (N = 512-128m), so scores/exp/PV skip ~15% of work; the causal
    boundary is a single [128,2,128] triangular-mask multiply per
    diagonal step (c >= p), instead of full-tile masks.
  - exp on the scalar engine straight out of PSUM, one instruction
    covering both heads (1/sqrt(hd) folded into Wq on the host).
  - PV uses ones-augmented V ([k,65] lhsT) so PSUM row 64 accumulates
    the softmax denominator Z alongside the 64 output dims.
  - normalization: Z rows leave PSUM through an ACT Ln pass (fused
    extract+log), 1/Z = exp(-ln Z) on ACT (one instr per q-tile),
    broadcast across 64 partitions with a K=97 sel-matmul, then one
    DVE multiply per (pair, half).
  - emission interleaves QKV(j+1) and proj(j-1) matmuls into
    attention(j)'s exp-gated stream so the PE stays dense and warm;
    activations are per-j tiles so interleaved phases share no tiles.
"""

import sys

sys.path.insert(0, "/opt/trn_rl_repo")

import ml_dtypes
import numpy as np

import concourse.bass as bass
import concourse.tile as tile
from concourse import bacc, mybir

F32 = mybir.dt.float32
BF16 = mybir.dt.bfloat16

B, T, D = 2, 2048, 1024
H, HD = 16, 64
N_CORES = 8
HPG = H // (N_CORES // B)  # heads per group = 4
GF = HPG * HD  # per-group feature width = 256
DT = 512  # t/q tile width
KT = 128  # k tile width
NJ = T // DT  # 4
ND = D // 128  # 8 contraction chunks

Exp = mybir.ActivationFunctionType.Exp
Ln = mybir.ActivationFunctionType.Ln


def build_program(num_devices=N_CORES):
    nc = bacc.Bacc(
        "TRN2", target_bir_lowering=False, debug=False, num_devices=num_devices
    )
    xT_d = nc.dram_tensor("xT", [D, T], BF16, kind="ExternalInput")
    wq_d = nc.dram_tensor("wq", [D, GF], BF16, kind="ExternalInput")
    wk_d = nc.dram_tensor("wk", [D, GF], BF16, kind="ExternalInput")
    wv_d = nc.dram_tensor("wv", [D, GF], BF16, kind="ExternalInput")
    wp_d = nc.dram_tensor("wp", [GF, D], BF16, kind="ExternalInput")
    bq_d = nc.dram_tensor("bq", [128, 2], F32, kind="ExternalInput")
    bk_d = nc.dram_tensor("bk", [128, 2], F32, kind="ExternalInput")
    bv_d = nc.dram_tensor("bv", [1, GF], BF16, kind="ExternalInput")
    ones_d = nc.dram_tensor("ones", [128, 128], BF16, kind="ExternalInput")
    msk_d = nc.dram_tensor("msk", [128, 2, KT], BF16, kind="ExternalInput")
    sel_d = nc.dram_tensor("sel", [128, 256], BF16, kind="ExternalInput")
    y_d = nc.dram_tensor("y", [T, D], BF16, kind="ExternalOutput")

    with tile.TileContext(nc) as tc:
        with (
            tc.tile_pool(name="singles", bufs=1) as singles,
            tc.tile_pool(name="ea", bufs=6) as e_pool,
            tc.tile_pool(name="rz", bufs=2) as rz_pool,
            tc.tile_pool(name="ysb", bufs=3) as y_pool,
            tc.tile_pool(name="tr", bufs=2, space="PSUM") as tr_pool,
            tc.tile_pool(name="sc", bufs=2, space="PSUM") as sc_pool,
            tc.tile_pool(name="pv", bufs=2, space="PSUM") as pv_pool,
        ):
            # ---- weights / constants resident in SBUF ----
            wq_sb = singles.tile([128, ND, GF], BF16)
            wk_sb = singles.tile([128, ND, GF], BF16)
            wv_sb = singles.tile([128, ND, GF], BF16)
            wp_sb = singles.tile([128, 2, D], BF16)
            bq_sb = singles.tile([128, 2], F32)
            bk_sb = singles.tile([128, 2], F32)
            bv_sb = singles.tile([1, GF], BF16)
            ones_sb = singles.tile([128, 128], BF16)
            msk_sb = singles.tile([128, 2, KT], BF16)
            sel_sb = singles.tile([128, 256], BF16)

            wq_v = wq_d.ap().rearrange("(o p) c -> p o c", p=128)
            wk_v = wk_d.ap().rearrange("(o p) c -> p o c", p=128)
            wv_v = wv_d.ap().rearrange("(o p) c -> p o c", p=128)
            wp_v = wp_d.ap().rearrange("(o p) n -> p o n", p=128)
            xT_v = xT_d.ap().rearrange("(o p) t -> p o t", p=128)

            # per-j activation tiles (distinct tiles -> no false deps
            # between interleaved phases)
            qT_t = [singles.tile([128, 2, DT], BF16, tag=f"qT{j}", name=f"qT{j}") for j in range(NJ)]
            kT_t = [singles.tile([128, 2, DT], BF16, tag=f"kT{j}", name=f"kT{j}") for j in range(NJ)]
            v_t = [
                singles.tile([128, DT // KT, HPG, HD + 1], BF16, tag=f"v{j}", name=f"v{j}")
                for j in range(NJ)
            ]
            o_t = [singles.tile([128, 2, DT], BF16, tag=f"oT{j}", name=f"oT{j}") for j in range(NJ)]
            xt_t = [
                singles.tile([128, ND, DT], BF16, tag=f"xt{j}", name=f"xt{j}")
                for j in range(NJ)
            ]

            # ---- early, chunked loads: first QKV matmul needs only
            # wq[d=0] + xt0[d=0] ----
            for d in range(ND):
                nc.sync.dma_start(xt_t[0][:, d, :], xT_v[:, d, 0:DT]).annotate("ld:xt0")
                nc.sync.dma_start(wq_sb[:, d, :], wq_v[:, d, :]).annotate("ld:wq")
                nc.sync.dma_start(wk_sb[:, d, :], wk_v[:, d, :]).annotate("ld:wk")
                nc.sync.dma_start(wv_sb[:, d, :], wv_v[:, d, :]).annotate("ld:wv")
            nc.sync.dma_start(bq_sb, bq_d.ap()).annotate("ld:b")
            nc.sync.dma_start(bk_sb, bk_d.ap()).annotate("ld:b")
            nc.sync.dma_start(bv_sb, bv_d.ap()).annotate("ld:b")
            nc.sync.dma_start(ones_sb, ones_d.ap()).annotate("ld:b")
            nc.sync.dma_start(msk_sb, msk_d.ap()).annotate("ld:b")
            nc.sync.dma_start(sel_sb, sel_d.ap()).annotate("ld:b")
            for ch in range(2):
                nc.sync.dma_start(wp_sb[:, ch, :], wp_v[:, ch, :]).annotate("ld:wp")
            ones1 = ones_sb[0:1, :]
            for j in range(NJ):
                nc.vector.tensor_copy(
                    out=v_t[j][:, :, :, HD],
                    in_=ones_sb[:, 0 : DT // KT * HPG].rearrange(
                        "p (a b) -> p a b", a=DT // KT
                    ),
                ).annotate("v:ones")

            def qkv_closures(j):
                """QKV production for t-tile j as a list of closures."""
                cls = []
                if j > 0:

                    def ldx(j=j):
                        for d in range(ND):
                            nc.sync.dma_start(
                                xt_t[j][:, d, :],
                                xT_v[:, d, j * DT : (j + 1) * DT],
                            ).annotate(f"ld:xt{j}")

                    cls.append(ldx)

                for w_sb, b_sb, dst in (
                    (wq_sb, bq_sb, qT_t[j]),
                    (wk_sb, bk_sb, kT_t[j]),
                ):

                    def qk(j=j, w_sb=w_sb, b_sb=b_sb, dst=dst):
                        for ch in range(2):
                            ps = tr_pool.tile([128, DT], F32, tag="tr")
                            csl = slice(ch * 128, ch * 128 + 128)
                            for d in range(ND):
                                nc.tensor.matmul(
                                    ps,
                                    w_sb[:, d, csl],
                                    xt_t[j][:, d, :],
                                    start=(d == 0),
                                    stop=(d == ND - 1),
                                ).annotate("mm:qk")
                            nc.vector.tensor_scalar_add(
                                out=dst[:, ch, :],
                                in0=ps,
                                scalar1=b_sb[:, ch : ch + 1],
                            ).annotate("cp:qk")

                    cls.append(qk)

                for t_ in range(DT // KT):

                    def vv(j=j, t_=t_):
                        ps = tr_pool.tile([128, DT], F32, tag="tr")
                        ssl = slice(t_ * 128, t_ * 128 + 128)
                        for d in range(ND):
                            nc.tensor.matmul(
                                ps[:, 0:GF],
                                xt_t[j][:, d, ssl],
                                wv_sb[:, d, :],
                                start=(d == 0),
                                stop=False,
                            ).annotate("mm:v")
                        nc.tensor.matmul(
                            ps[:, 0:GF], ones1, bv_sb, start=False, stop=True
                        ).annotate("mm:vb")
                        nc.vector.tensor_copy(
                            out=v_t[j][:, t_, :, 0:HD],
                            in_=ps[:, 0:GF].rearrange("p (h c) -> p h c", c=HD),
                        ).annotate("cp:v")

                    cls.append(vv)
                return cls

            def attn_closures(j):
                """Attention for q-tile j: per-(pair,kt) closures plus a
                normalize closure per pair."""
                nk = 4 * (j + 1)
                cls = []
                zz = rz_pool.tile([128, DT], F32, tag="zz", name=f"zz{j}")
                nc.vector.memset(zz, 0.0)
                for pair in range(2):
                    pvA = pv_pool.tile([HD + 1, DT], F32, tag="pv")
                    pvB = pv_pool.tile([HD + 1, DT], F32, tag="pv")

                    for kt in range(nk):
                        m = kt - 4 * j
                        qoff = 128 * m if m >= 0 else 0

                        def step(j=j, pair=pair, kt=kt, m=m, qoff=qoff,
                                 pvA=pvA, pvB=pvB, nk=nk):
                            jk, km = kt // (DT // KT), kt % (DT // KT)
                            ksl = slice(km * KT, km * KT + KT)
                            psc = sc_pool.tile([128, 2, DT], F32, tag="sc")
                            nc.tensor.matmul(
                                psc[:, 0, qoff:],
                                kT_t[jk][0:64, pair, ksl],
                                qT_t[j][0:64, pair, qoff:],
                                start=True,
                                stop=True,
                            ).annotate("mm:s")
                            nc.tensor.matmul(
                                psc[:, 1, qoff:],
                                kT_t[jk][64:128, pair, ksl],
                                qT_t[j][64:128, pair, qoff:],
                                start=True,
                                stop=True,
                            ).annotate("mm:s")
                            e = e_pool.tile([128, 2, DT], BF16, tag="e")
                            nc.scalar.activation(
                                out=e[:, :, qoff:], in_=psc[:, :, qoff:], func=Exp
                            ).annotate("exp")
                            if m >= 0:  # diagonal tile: causal boundary mask
                                nc.vector.tensor_mul(
                                    out=e[:, :, qoff : qoff + KT],
                                    in0=e[:, :, qoff : qoff + KT],
                                    in1=msk_sb,
                                ).annotate("mask")
                            nc.tensor.matmul(
                                pvA[:, qoff:],
                                v_t[jk][:, km, 2 * pair, :],
                                e[:, 0, qoff:],
                                start=(kt == 0),
                                stop=(kt == nk - 1),
                            ).annotate("mm:pv")
                            nc.tensor.matmul(
                                pvB[:, qoff:],
                                v_t[jk][:, km, 2 * pair + 1, :],
                                e[:, 1, qoff:],
                                start=(kt == 0),
                                stop=(kt == nk - 1),
                            ).annotate("mm:pv")

                        cls.append(step)

                    def zcp(j=j, pair=pair, pvA=pvA, pvB=pvB, zz=zz):
                        # move Z rows out through an ACT Ln (fused extract+log)
                        # and evacuate the unnormalized outputs NOW so the pv
                        # PSUM banks release quickly
                        for half, pv in ((0, pvA), (1, pvB)):
                            row = 32 * (2 * pair + half)
                            nc.scalar.activation(
                                out=zz[row : row + 1, :],
                                in_=pv[HD : HD + 1, :],
                                func=Ln,
                            ).annotate("lnz")
                            osl = o_t[j][half * 64 : half * 64 + 64, pair, :]
                            nc.vector.tensor_copy(
                                out=osl, in_=pv[0:HD, :]
                            ).annotate("cp:o")

                    cls.append(zcp)

                def norm(j=j, zz=zz):
                    zr = rz_pool.tile([128, DT], BF16, tag="zr")
                    with nc.allow_low_precision(reason="bf16 rz for bcast"):
                        nc.scalar.activation(
                            out=zr[0:97, :], in_=zz[0:97, :], func=Exp, scale=-1.0
                        ).annotate("rz")
                    for pair in range(2):
                        bz = tr_pool.tile([128, DT], F32, tag="tr")
                        nc.tensor.matmul(
                            bz,
                            sel_sb[0:97, pair * 128 : (pair + 1) * 128],
                            zr[0:97, :],
                            start=True,
                            stop=True,
                        ).annotate("mm:bz")
                        for half in range(2):
                            osl = o_t[j][half * 64 : half * 64 + 64, pair, :]
                            nc.vector.tensor_mul(
                                out=osl, in0=osl, in1=bz[half * 64 : half * 64 + 64, :]
                            ).annotate("mul:o")

                return cls, norm

            def proj_closures(j):
                cls = []
                for t_ in range(DT // KT):

                    def pj(j=j, t_=t_):
                        t0 = j * DT + t_ * 128
                        lsl = slice(t_ * 128, t_ * 128 + 128)
                        ysb = y_pool.tile([128, D], BF16, tag="y")
                        for n in range(2):
                            ps = tr_pool.tile([128, DT], F32, tag="tr")
                            nsl = slice(n * DT, n * DT + DT)
                            for ch in range(2):
                                nc.tensor.matmul(
                                    ps,
                                    o_t[j][:, ch, lsl],
                                    wp_sb[:, ch, nsl],
                                    start=(ch == 0),
                                    stop=(ch == 1),
                                ).annotate("mm:p")
                            nc.vector.tensor_copy(out=ysb[:, nsl], in_=ps).annotate(
                                "cp:y"
                            )
                        nc.sync.dma_start(y_d.ap()[t0 : t0 + 128, :], ysb).annotate(
                            "st:y"
                        )

                    cls.append(pj)
                return cls

            # ---- emission: QKV(0) plain, then per j interleave
            # attention(j) with QKV(j+1) + proj(j-1) ----
            with nc.allow_low_precision(reason="bf16 activations"):
                for c in qkv_closures(0):
                    c()
                prev_norm = None
                for j in range(NJ):
                    attn, norm = attn_closures(j)
                    fill = []
                    if prev_norm is not None:
                        fill.append(prev_norm)
                    if j + 1 < NJ:
                        fill += qkv_closures(j + 1)
                    if j >= 1:
                        fill += proj_closures(j - 1)
                    prev_norm = norm
                    done = 0
                    for i, c in enumerate(attn):
                        c()
                        want = (i + 1) * len(fill) // len(attn)
                        while done < want:
                            fill[done]()
                            done += 1
                    while done < len(fill):
                        fill[done]()
                        done += 1
                prev_norm()
                for c in proj_closures(NJ - 1):
                    c()

    return nc


def shard_inputs(x, W_kqv, b_kqv, W_proj, b_proj):
    """Build the 8 per-core input maps (host-side layout transforms)."""
    scale = 1.0 / np.sqrt(np.float32(HD))
    bf = lambda a: np.ascontiguousarray(a).astype(ml_dtypes.bfloat16)
    in_maps = []
    for c in range(N_CORES):
        b = c // (N_CORES // B)
        g = c % (N_CORES // B)
        gsl = slice(g * GF, (g + 1) * GF)
        wq = np.ascontiguousarray(W_kqv[:, gsl]) * scale
        wk = W_kqv[:, D + g * GF : D + (g + 1) * GF]
        wv = W_kqv[:, 2 * D + g * GF : 2 * D + (g + 1) * GF]
        bq = (b_kqv[gsl] * scale).reshape(2, 128).T
        bk = b_kqv[D + g * GF : D + (g + 1) * GF].reshape(2, 128).T
        bv = b_kqv[2 * D + g * GF : 2 * D + (g + 1) * GF].reshape(1, GF)
        in_maps.append(
            {
                "xT": bf(np.asarray(x[b]).T),
                "wq": bf(wq),
                "wk": bf(wk),
                "wv": bf(wv),
                "wp": bf(W_proj[gsl, :]),
                "bq": np.ascontiguousarray(bq).astype(np.float32),
                "bk": np.ascontiguousarray(bk).astype(np.float32),
                "bv": bf(bv),
                "ones": np.ones((128, 128), dtype=ml_dtypes.bfloat16),
                "msk": _mask_tiles(),
                "sel": _sel_tiles(),
            }
        )
    return in_maps


def _sel_tiles():
    sel = np.zeros((128, 256), dtype=ml_dtypes.bfloat16)
    for p in range(2):
        for c in range(128):
            sel[32 * (2 * p + (c >= 64)), p * 128 + c] = 1.0
    return sel


def _mask_tiles():
    # triangular causal boundary for a diagonal [128k x 128q] corner:
    # keep where q_local >= k_local (c >= p), duplicated for both heads
    p = np.arange(128)[:, None]
    c = np.arange(KT)[None, :]
    m = (c >= p).astype(ml_dtypes.bfloat16)
    return np.ascontiguousarray(np.stack([m, m], axis=1))


def gather_outputs(results, b_proj):
    out = np.zeros((B, T, D), dtype=np.float32)
    for c in range(N_CORES):
        out[c // (N_CORES // B)] += np.asarray(results[c]["y"], dtype=np.float32)
    out += b_proj[None, None, :].astype(np.float32)
    return out


_NC_CACHE = {}


def _get_program():
    if "nc" not in _NC_CACHE:
        nc = build_program()
        nc.finalize()  # runs Bacc passes (reg alloc, wait splitting)
        _NC_CACHE["nc"] = nc
    return _NC_CACHE["nc"]


def run(inputs, trace=False):
    """Run on the 8 NeuronCores; returns (out, BassKernelResults)."""
    from concourse import bass_utils

    nc = _get_program()
    in_maps = shard_inputs(**inputs)
    res = bass_utils.run_bass_kernel_spmd(
        nc,
        in_maps,
        core_ids=list(range(N_CORES)),
        trace=trace,
        trace_cores=list(range(N_CORES)) if trace else None,
    )
    out = gather_outputs(res.results, inputs["b_proj"])
    return out, res


def kernel(**inputs):
    out, _ = run(inputs, trace=False)
    return out


# revision 14
# speedup vs baseline: 1.4677x; 1.1393x over previous
"""Causal MHA on 8 trn2 NeuronCores.

Sharding: core c handles batch b = c // 4 and head group g = c % 4
(heads 4g..4g+3).  Megatron-style TP: W_kqv column-split per head
group, W_proj row-split; the row-parallel all-reduce (sum of the 4
head-group partials per batch) happens on the host at gather time.

Per-core program (bf16 matmul operands, fp32 PSUM accumulation):
  - qT,kT produced directly in [feat, T] layout (lhsT=W tiles, rhs=xT
    tiles), v in [T, feat] layout, so no on-device transposes.
  - scores computed transposed, sT[k,q]; two heads packed into the PE
    array rows (K=64 each) via base-partition 0/64 -> concurrent MMs
    into the two banks of one [128,2,512] f32 PSUM tile.
  - causal trim: diagonal k-tiles only compute the valid q-range
    (N = 512-128m), so scores/exp/PV skip ~15% of work; the causal
    boundary is a single [128,2,128] triangular-mask multiply per
    diagonal step (c >= p), instead of full-tile masks.
  - exp on the scalar engine straight out of PSUM, one instruction
    covering both heads (1/sqrt(hd) folded into Wq on the host).
  - PV uses ones-augmented V ([k,65] lhsT) so PSUM row 64 accumulates
    the softmax denominator Z alongside the 64 output dims.
  - normalization: Z rows leave PSUM through an ACT Ln pass (fused
    extract+log), 1/Z = exp(-ln Z) on ACT (one instr per q-tile),
    broadcast across 64 partitions with a K=97 sel-matmul, then one
    DVE multiply per (pair, half).
  - emission interleaves QKV(j+1) and proj(j-1) matmuls into
    attention(j)'s exp-gated stream so the PE stays dense and warm;
    activations are per-j tiles so interleaved phases share no tiles.
"""

import sys

sys.path.insert(0, "/opt/trn_rl_repo")

import ml_dtypes
import numpy as np

import concourse.bass as bass
import concourse.tile as tile
from concourse import bacc, mybir

F32 = mybir.dt.float32
F32R = mybir.dt.float32r
BF16 = mybir.dt.bfloat16

B, T, D = 2, 2048, 1024
H, HD = 16, 64
N_CORES = 8
HPG = H // (N_CORES // B)  # heads per group = 4
GF = HPG * HD  # per-group feature width = 256
DT = 512  # t/q tile width
KT = 128  # k tile width
NJ = T // DT  # 4
ND = D // 128  # 8 contraction chunks

Exp = mybir.ActivationFunctionType.Exp
Ln = mybir.ActivationFunctionType.Ln


def build_program(num_devices=N_CORES):
    nc = bacc.Bacc(
        "TRN2", target_bir_lowering=False, debug=False, num_devices=num_devices
    )
    xT_d = nc.dram_tensor("xT", [D, T], BF16, kind="ExternalInput")
    wq_d = nc.dram_tensor("wq", [D, GF], BF16, kind="ExternalInput")
    wk_d = nc.dram_tensor("wk", [D, GF], BF16, kind="ExternalInput")
    wv_d = nc.dram_tensor("wv", [D, GF], BF16, kind="ExternalInput")
    wp_d = nc.dram_tensor("wp", [GF, D], BF16, kind="ExternalInput")
    bq_d = nc.dram_tensor("bq", [128, 2], F32, kind="ExternalInput")
    bk_d = nc.dram_tensor("bk", [128, 2], F32, kind="ExternalInput")
    bv_d = nc.dram_tensor("bv", [1, GF], BF16, kind="ExternalInput")
    ones_d = nc.dram_tensor("ones", [128, 128], BF16, kind="ExternalInput")
    msk_d = nc.dram_tensor("msk", [128, 2, KT], BF16, kind="ExternalInput")
    sel_d = nc.dram_tensor("sel", [128, 256], BF16, kind="ExternalInput")
    y_d = nc.dram_tensor("y", [T, D], BF16, kind="ExternalOutput")

    with tile.TileContext(nc) as tc:
        with (
            tc.tile_pool(name="singles", bufs=1) as singles,
            tc.tile_pool(name="ea", bufs=6) as e_pool,
            tc.tile_pool(name="rz", bufs=2) as rz_pool,
            tc.tile_pool(name="ysb", bufs=3) as y_pool,
            tc.tile_pool(name="tr", bufs=2, space="PSUM") as tr_pool,
            tc.tile_pool(name="sc", bufs=2, space="PSUM") as sc_pool,
            tc.tile_pool(name="pv", bufs=2, space="PSUM") as pv_pool,
        ):
            # ---- weights / constants resident in SBUF ----
            wq_sb = singles.tile([128, ND, GF], BF16)
            wk_sb = singles.tile([128, ND, GF], BF16)
            wv_sb = singles.tile([128, ND, GF], BF16)
            wp_sb = singles.tile([128, 2, D], BF16)
            bq_sb = singles.tile([128, 2], F32)
            bk_sb = singles.tile([128, 2], F32)
            bv_sb = singles.tile([1, GF], BF16)
            ones_sb = singles.tile([128, 128], BF16)
            msk_sb = singles.tile([128, 2, KT], BF16)
            sel_sb = singles.tile([128, 256], BF16)

            wq_v = wq_d.ap().rearrange("(o p) c -> p o c", p=128)
            wk_v = wk_d.ap().rearrange("(o p) c -> p o c", p=128)
            wv_v = wv_d.ap().rearrange("(o p) c -> p o c", p=128)
            wp_v = wp_d.ap().rearrange("(o p) n -> p o n", p=128)
            xT_v = xT_d.ap().rearrange("(o p) t -> p o t", p=128)

            # per-j activation tiles (distinct tiles -> no false deps
            # between interleaved phases)
            qT_t = [singles.tile([128, 2, DT], BF16, tag=f"qT{j}", name=f"qT{j}") for j in range(NJ)]
            kT_t = [singles.tile([128, 2, DT], BF16, tag=f"kT{j}", name=f"kT{j}") for j in range(NJ)]
            v_t = [
                singles.tile([128, DT // KT, HPG, HD + 1], BF16, tag=f"v{j}", name=f"v{j}")
                for j in range(NJ)
            ]
            o_t = [singles.tile([128, 2, DT], BF16, tag=f"oT{j}", name=f"oT{j}") for j in range(NJ)]
            xt_t = [
                singles.tile([128, ND, DT], BF16, tag=f"xt{j}", name=f"xt{j}")
                for j in range(NJ)
            ]

            # ---- early, chunked loads: first QKV matmul needs only
            # wq[d=0] + xt0[d=0] ----
            for d in range(ND):
                nc.sync.dma_start(xt_t[0][:, d, :], xT_v[:, d, 0:DT]).annotate("ld:xt0")
                nc.sync.dma_start(wq_sb[:, d, :], wq_v[:, d, :]).annotate("ld:wq")
                nc.sync.dma_start(wk_sb[:, d, :], wk_v[:, d, :]).annotate("ld:wk")
                nc.sync.dma_start(wv_sb[:, d, :], wv_v[:, d, :]).annotate("ld:wv")
            nc.sync.dma_start(bq_sb, bq_d.ap()).annotate("ld:b")
            nc.sync.dma_start(bk_sb, bk_d.ap()).annotate("ld:b")
            nc.sync.dma_start(bv_sb, bv_d.ap()).annotate("ld:b")
            nc.sync.dma_start(ones_sb, ones_d.ap()).annotate("ld:b")
            nc.sync.dma_start(msk_sb, msk_d.ap()).annotate("ld:b")
            nc.sync.dma_start(sel_sb, sel_d.ap()).annotate("ld:b")
            for ch in range(2):
                nc.sync.dma_start(wp_sb[:, ch, :], wp_v[:, ch, :]).annotate("ld:wp")
            ones1 = ones_sb[0:1, :]
            for j in range(NJ):
                nc.vector.tensor_copy(
                    out=v_t[j][:, :, :, HD],
                    in_=ones_sb[:, 0 : DT // KT * HPG].rearrange(
                        "p (a b) -> p a b", a=DT // KT
                    ),
                ).annotate("v:ones")

            def qkv_closures(j):
                """QKV production for t-tile j as a list of closures."""
                cls = []
                if j > 0:

                    def ldx(j=j):
                        for d in range(ND):
                            nc.sync.dma_start(
                                xt_t[j][:, d, :],
                                xT_v[:, d, j * DT : (j + 1) * DT],
                            ).annotate(f"ld:xt{j}")

                    cls.append(ldx)

                for w_sb, b_sb, dst in (
                    (wq_sb, bq_sb, qT_t[j]),
                    (wk_sb, bk_sb, kT_t[j]),
                ):

                    def qk(j=j, w_sb=w_sb, b_sb=b_sb, dst=dst):
                        for ch in range(2):
                            ps = tr_pool.tile([128, DT], F32, tag="tr")
                            csl = slice(ch * 128, ch * 128 + 128)
                            for d in range(ND):
                                nc.tensor.matmul(
                                    ps,
                                    w_sb[:, d, csl],
                                    xt_t[j][:, d, :],
                                    start=(d == 0),
                                    stop=(d == ND - 1),
                                ).annotate("mm:qk")
                            nc.vector.tensor_scalar_add(
                                out=dst[:, ch, :],
                                in0=ps,
                                scalar1=b_sb[:, ch : ch + 1],
                            ).annotate("cp:qk")

                    cls.append(qk)

                for t_ in range(DT // KT):

                    def vv(j=j, t_=t_):
                        ps = tr_pool.tile([128, DT], F32, tag="tr")
                        ssl = slice(t_ * 128, t_ * 128 + 128)
                        for d in range(ND):
                            nc.tensor.matmul(
                                ps[:, 0:GF],
                                xt_t[j][:, d, ssl],
                                wv_sb[:, d, :],
                                start=(d == 0),
                                stop=False,
                            ).annotate("mm:v")
                        nc.tensor.matmul(
                            ps[:, 0:GF], ones1, bv_sb, start=False, stop=True
                        ).annotate("mm:vb")
                        nc.scalar.copy(
                            out=v_t[j][:, t_, :, 0:HD],
                            in_=ps[:, 0:GF].rearrange("p (h c) -> p h c", c=HD),
                        ).annotate("cp:v")

                    cls.append(vv)
                return cls

            def attn_closures(j):
                """Attention for q-tile j: per-(pair,kt) closures plus a
                normalize closure per pair."""
                nk = 4 * (j + 1)
                cls = []
                zz = rz_pool.tile([128, DT], F32, tag="zz", name=f"zz{j}")
                nc.vector.memset(zz, 1.0)
                for pair in range(2):
                    pvA = pv_pool.tile([HD + 1, DT], F32, tag="pv")
                    pvB = pv_pool.tile([HD + 1, DT], F32, tag="pv")

                    for kt in range(nk):
                        m = kt - 4 * j
                        qoff = 128 * m if m >= 0 else 0

                        def step(j=j, pair=pair, kt=kt, m=m, qoff=qoff,
                                 pvA=pvA, pvB=pvB, nk=nk):
                            jk, km = kt // (DT // KT), kt % (DT // KT)
                            ksl = slice(km * KT, km * KT + KT)
                            psc = sc_pool.tile([128, 2, DT], F32, tag="sc")
                            nc.tensor.matmul(
                                psc[:, 0, qoff:],
                                kT_t[jk][0:64, pair, ksl],
                                qT_t[j][0:64, pair, qoff:],
                                start=True,
                                stop=True,
                            ).annotate("mm:s")
                            nc.tensor.matmul(
                                psc[:, 1, qoff:],
                                kT_t[jk][64:128, pair, ksl],
                                qT_t[j][64:128, pair, qoff:],
                                start=True,
                                stop=True,
                            ).annotate("mm:s")
                            e = e_pool.tile([128, 2, DT], BF16, tag="e")
                            nc.scalar.activation(
                                out=e[:, :, qoff:], in_=psc[:, :, qoff:], func=Exp
                            ).annotate("exp")
                            if m >= 0:  # diagonal tile: causal boundary mask
                                nc.vector.tensor_mul(
                                    out=e[:, :, qoff : qoff + KT],
                                    in0=e[:, :, qoff : qoff + KT],
                                    in1=msk_sb,
                                ).annotate("mask")
                            nc.tensor.matmul(
                                pvA[:, qoff:],
                                v_t[jk][:, km, 2 * pair, :],
                                e[:, 0, qoff:],
                                start=(kt == 0),
                                stop=(kt == nk - 1),
                            ).annotate("mm:pv")
                            nc.tensor.matmul(
                                pvB[:, qoff:],
                                v_t[jk][:, km, 2 * pair + 1, :],
                                e[:, 1, qoff:],
                                start=(kt == 0),
                                stop=(kt == nk - 1),
                            ).annotate("mm:pv")

                        cls.append(step)

                    def zcp(j=j, pair=pair, pvA=pvA, pvB=pvB, zz=zz):
                        # copy Z rows and the unnormalized outputs NOW so the
                        # pv PSUM banks release quickly
                        for half, pv in ((0, pvA), (1, pvB)):
                            row = 32 * (2 * pair + half)
                            nc.vector.tensor_copy(
                                out=zz[row : row + 1, :],
                                in_=pv[HD : HD + 1, :],
                            ).annotate("zcp")
                            osl = o_t[j][half * 64 : half * 64 + 64, pair, :]
                            nc.vector.tensor_copy(
                                out=osl, in_=pv[0:HD, :]
                            ).annotate("cp:o")

                    cls.append(zcp)

                def norm(j=j, zz=zz):
                    zr = rz_pool.tile([128, DT], F32, tag="zr")
                    nc.vector.reciprocal_approx_fast(
                        out=zr[0:97, :], in_=zz[0:97, :]
                    ).annotate("rz")
                    zrb = rz_pool.tile([128, DT], BF16, tag="zrb")
                    nc.vector.tensor_copy(
                        out=zrb[0:97, :], in_=zr[0:97, :]
                    ).annotate("rzb")
                    for pair in range(2):
                        bz = tr_pool.tile([128, DT], F32, tag="tr")
                        nc.tensor.matmul(
                            bz,
                            sel_sb[0:97, pair * 128 : (pair + 1) * 128],
                            zrb[0:97, :],
                            start=True,
                            stop=True,
                        ).annotate("mm:bz")
                        for half in range(2):
                            osl = o_t[j][half * 64 : half * 64 + 64, pair, :]
                            nc.vector.tensor_mul(
                                out=osl, in0=osl, in1=bz[half * 64 : half * 64 + 64, :]
                            ).annotate("mul:o")

                return cls, norm

            def proj_closures(j):
                cls = []
                for t_ in range(DT // KT):

                    def pj(j=j, t_=t_):
                        t0 = j * DT + t_ * 128
                        lsl = slice(t_ * 128, t_ * 128 + 128)
                        ysb = y_pool.tile([128, D], BF16, tag="y")
                        for n in range(2):
                            ps = tr_pool.tile([128, DT], F32, tag="tr")
                            nsl = slice(n * DT, n * DT + DT)
                            for ch in range(2):
                                nc.tensor.matmul(
                                    ps,
                                    o_t[j][:, ch, lsl],
                                    wp_sb[:, ch, nsl],
                                    start=(ch == 0),
                                    stop=(ch == 1),
                                ).annotate("mm:p")
                            nc.vector.tensor_copy(out=ysb[:, nsl], in_=ps).annotate(
                                "cp:y"
                            )
                        nc.sync.dma_start(y_d.ap()[t0 : t0 + 128, :], ysb).annotate(
                            "st:y"
                        )

                    cls.append(pj)
                return cls

            # ---- emission: QKV(0) plain, then per j interleave
            # attention(j) with QKV(j+1) + proj(j-1) ----
            with nc.allow_low_precision(reason="bf16 activations"):
                for c in qkv_closures(0):
                    c()
                prev_norm = None
                for j in range(NJ):
                    attn, norm = attn_closures(j)
                    fill = []
                    if prev_norm is not None:
                        fill.append(prev_norm)
                    if j + 1 < NJ:
                        fill += qkv_closures(j + 1)
                    if j >= 1:
                        fill += proj_closures(j - 1)
                    prev_norm = norm
                    done = 0
                    for i, c in enumerate(attn):
                        c()
                        want = (i + 1) * len(fill) // len(attn)
                        while done < want:
                            fill[done]()
                            done += 1
                    while done < len(fill):
                        fill[done]()
                        done += 1
                prev_norm()
                for c in proj_closures(NJ - 1):
                    c()

    return nc


def shard_inputs(x, W_kqv, b_kqv, W_proj, b_proj):
    """Build the 8 per-core input maps (host-side layout transforms)."""
    scale = 1.0 / np.sqrt(np.float32(HD))
    bf = lambda a: np.ascontiguousarray(a).astype(ml_dtypes.bfloat16)
    in_maps = []
    for c in range(N_CORES):
        b = c // (N_CORES // B)
        g = c % (N_CORES // B)
        gsl = slice(g * GF, (g + 1) * GF)
        wq = np.ascontiguousarray(W_kqv[:, gsl]) * scale
        wk = W_kqv[:, D + g * GF : D + (g + 1) * GF]
        wv = W_kqv[:, 2 * D + g * GF : 2 * D + (g + 1) * GF]
        bq = (b_kqv[gsl] * scale).reshape(2, 128).T
        bk = b_kqv[D + g * GF : D + (g + 1) * GF].reshape(2, 128).T
        bv = b_kqv[2 * D + g * GF : 2 * D + (g + 1) * GF].reshape(1, GF)
        in_maps.append(
            {
                "xT": bf(np.asarray(x[b]).T),
                "wq": bf(wq),
                "wk": bf(wk),
                "wv": bf(wv),
                "wp": bf(W_proj[gsl, :]),
                "bq": np.ascontiguousarray(bq).astype(np.float32),
                "bk": np.ascontiguousarray(bk).astype(np.float32),
                "bv": bf(bv),
                "ones": np.ones((128, 128), dtype=ml_dtypes.bfloat16),
                "msk": _mask_tiles(),
                "sel": _sel_tiles(),
            }
        )
    return in_maps


def _sel_tiles():
    sel = np.zeros((128, 256), dtype=ml_dtypes.bfloat16)
    for p in range(2):
        for c in range(128):
            sel[32 * (2 * p + (c >= 64)), p * 128 + c] = 1.0
    return sel


def _mask_tiles():
    # triangular causal boundary for a diagonal [128k x 128q] corner:
    # keep where q_local >= k_local (c >= p), duplicated for both heads
    p = np.arange(128)[:, None]
    c = np.arange(KT)[None, :]
    m = (c >= p).astype(ml_dtypes.bfloat16)
    return np.ascontiguousarray(np.stack([m, m], axis=1))


def gather_outputs(results, b_proj):
    out = np.zeros((B, T, D), dtype=np.float32)
    for c in range(N_CORES):
        out[c // (N_CORES // B)] += np.asarray(results[c]["y"], dtype=np.float32)
    out += b_proj[None, None, :].astype(np.float32)
    return out


_NC_CACHE = {}


def _get_program():
    if "nc" not in _NC_CACHE:
        nc = build_program()
        nc.finalize()  # runs Bacc passes (reg alloc, wait splitting)
        _NC_CACHE["nc"] = nc
    return _NC_CACHE["nc"]


def run(inputs, trace=False):
    """Run on the 8 NeuronCores; returns (out, BassKernelResults)."""
    from concourse import bass_utils

    nc = _get_program()
    in_maps = shard_inputs(**inputs)
    res = bass_utils.run_bass_kernel_spmd(
        nc,
        in_maps,
        core_ids=list(range(N_CORES)),
        trace=trace,
        trace_cores=list(range(N_CORES)) if trace else None,
    )
    out = gather_outputs(res.results, inputs["b_proj"])
    return out, res


def kernel(**inputs):
    out, _ = run(inputs, trace=False)
    return out
